# revision 50
# baseline (speedup 1.0000x reference)
# Bass/Trainium2 kernel for nn_ABELSpline (embedding_lookup via implicit one-hot matmul).
#
# Math: out[b,o] = sum_d sum_r B(256*x[b,d] + 3 - r) * table[d*259 + r, o]
#   where B is the cubic B-spline basis on [0,4). The gather/multiply/reduce of the
#   reference is exactly a dense matmul with W[r, b] = B(...) built on the fly:
#     B(y) = relu(2-u)^3/6 - (2/3)*relu(1-u)^3,  u = |y-2|
#
# Orientation: stationary = table chunk [128 rows, <=128 cols], moving = W
# [128 rows, 512 batch]; psum chunk c accumulates [cols_c, 512 batch] over all
# 64 dim-halves + edge rows. 293 matmuls of N=512 (vs ~1040 in the
# batch-stationary orientation) cut PE sequencer issue work nearly in half
# while keeping PE engine time at the dense-matmul floor (~62us).
#
# W build (per core, 16 (g,h) pairs of [128 rows, 2048]):
#   - 13 pairs on DVE: two fused custom ops (relu-min cube) -> ~58us
#   - 3 pairs ((7,0),(7,1),(6,1)) on ScalarE activations + Pool combines
# Tables are pre-converted to bf16 on host.
#
# Sharding: cores form a 2x4 grid: batch halves (512) x dim quarters (32 dims).
# Partials [576 cols, 512 batch] bf16 ReduceScatter (cc_dim=Free) over the 4
# dim-parallel cores -> each core owns a 128-batch slice [576, 128]; the
# anti-symmetric-exponential tail is exp on ScalarE + a [128,64] select-matmul
# partition reduction on PE; each core outputs out^T [64, 128] f32 and the
# host transposes/assembles.

import numpy as np

BATCH = 1024
INPUT_DIM = 128
DENSITY = 259
OUTPUT_DIM = 64
NUM_EXPS = 4
INDIRECT_DIM = 2 * NUM_EXPS * OUTPUT_DIM  # 512
NCOLS = INDIRECT_DIM + OUTPUT_DIM  # 576
NCORES = 8
DGROUPS = 4                 # dim-parallel cores per batch half
BGROUPS = NCORES // DGROUPS  # 2 batch halves
DPC = INPUT_DIM // DGROUPS  # dims per core = 32
BL = BATCH // BGROUPS       # local batch = 512
GDIMS = 4                   # dims per W build group
NGROUPS = DPC // GDIMS      # 8
MAIN_ROWS = 256
EDGE_ROWS = DENSITY - MAIN_ROWS  # 3
EP = DPC * EDGE_ROWS        # 96 edge partitions
SCALE = float(DENSITY - 3)  # 256
K1 = float((1.0 / 6.0) ** (1.0 / 3.0))
K2 = float((4.0 / 6.0) ** (1.0 / 3.0))
MBL = BL // 128             # psum batch chunks = 4
X_I16 = False               # ship x as int16 round(x * XQ) (halves x DMA)
XQ = 32767.0 if X_I16 else 1.0

_OPS = None
_NC = None


def _register_ops():
    global _OPS
    if _OPS is not None:
        return _OPS
    from concourse.dve_spec import Spec, Src0, Src1, C0, C1, C2, Bin, relu, sq, minn, lower
    from concourse.dve_uop import AluOp, DveOpSpec
    from concourse import dve_ops
    from concourse.dve_ops import DveOp

    def relu3_body():
        # relu(min(C0 - m, m + C1))^3 with m = Src0*C2
        m = Src0 * C2
        a = Bin(AluOp.SUBTRACT, C0, m)
        b = Bin(AluOp.ADD, m, C1)
        r = relu(minn(a, b))
        return r * sq(r)

    def ref_a(in0, in1, s0, s1, imm2):
        t = np.maximum(np.minimum(s0 - in0 * imm2, in0 * imm2 + s1), 0.0)
        return (t ** 3).astype(np.float32)

    def ref_b(in0, in1, s0, s1, imm2):
        t = np.maximum(np.minimum(s0 - in0 * imm2, in0 * imm2 + s1), 0.0)
        return (in1 - t ** 3).astype(np.float32)

    spec_a = Spec(body=relu3_body(), reference=ref_a)
    spec_b = Spec(body=Bin(AluOp.SUBTRACT, Src1, relu3_body()), reference=ref_b)

    ops = []
    for name, spec in (("SPLINE3A_ANT", spec_a), ("SPLINE3B_ANT", spec_b)):
        if name in dve_ops._SUB_OPCODE_FOR_NAME:
            ops.append(next(o for o in dve_ops.OPS if o.name == name))
            continue
        row = max(dve_ops._SUB_OPCODE_FOR_NAME.values()) + 1
        assert row < 0x20
        uops = lower(spec, ver="v3")
        rd1 = name.endswith("B_ANT")
        sha = DveOpSpec(name=name, opcode=row, uops=uops, rd1_en=rd1).sha("v3")
        op = DveOp(name, spec, subdim=False, uops_sha={"v3": sha})
        dve_ops.OPS.append(op)
        dve_ops._SUB_OPCODE_FOR_NAME[name] = row
        dve_ops.CUSTOM_DVE_SPECS[name] = spec
        ops.append(op)
    _OPS = tuple(ops)
    return _OPS


def _bias_consts():
    # [128, 16] f32 per-partition scalars.
    # cols 4h+{0..3}: main half h (r = 128h + p): K1*(r+1), K1*(3-r), K2*r, K2*(2-r)
    # cols 8..11: edge (r = 256 + p%3)
    # cols 12/13: SP-pipe Abs bias 1-r for half h (arg = 256x - (r-1))
    b = np.zeros((128, 16), np.float32)
    p = np.arange(128, dtype=np.float64)
    for h in (0, 1):
        r = 128 * h + p
        b[:, 4 * h + 0] = K1 * (r + 1)
        b[:, 4 * h + 1] = K1 * (3 - r)
        b[:, 4 * h + 2] = K2 * r
        b[:, 4 * h + 3] = K2 * (2 - r)
        b[:, 12 + h] = 1.0 - r
    b[:, 14] = 2.0 * K1
    b[:, 15] = K2
    re = 256 + (p % 3)
    b[:, 8] = K1 * (re + 1)
    b[:, 9] = K1 * (3 - re)
    b[:, 10] = K2 * re
    b[:, 11] = K2 * (2 - re)
    return b


def _wpat_const():
    w = np.array([1.0 / (i + 1) ** 2 for i in range(NUM_EXPS)], np.float32)
    row = np.concatenate([w, -w])  # [8]
    return np.tile(row, (128, OUTPUT_DIM)).astype(np.float32)  # [128, 512]


def _build(skip=(), loop_reps=0, reps=1):
    global _NC
    if _NC is not None and not skip and not loop_reps and reps == 1:
        return _NC
    import contextlib
    import concourse.bass as bass
    import concourse.bacc as bacc
    import concourse.tile as tile
    import concourse.mybir as mybir

    OP_A, OP_B = _register_ops()
    f32 = mybir.dt.float32
    bf16 = mybir.dt.bfloat16
    AF = mybir.ActivationFunctionType

    nc = bacc.Bacc("TRN2", target_bir_lowering=False, debug=False, num_devices=NCORES)

    i16 = mybir.dt.int16 if X_I16 else mybir.dt.float32
    xt_d = nc.dram_tensor("xt", [DPC, BL], i16, kind="ExternalInput")
    xe_d = nc.dram_tensor("xe", [EP, BL], i16, kind="ExternalInput")
    # tmain pre-permuted on host (bf16): [128, (2*DPC)*NCOLS], chunk k=2*di+h at cols k*NCOLS
    tmain_d = nc.dram_tensor("tmain", [128, 2 * DPC * NCOLS], bf16, kind="ExternalInput")
    tedge_d = nc.dram_tensor("tedge", [EP, NCOLS], bf16, kind="ExternalInput")
    bias_d = nc.dram_tensor("bias", [128, 16], f32, kind="ExternalInput")
    wpat_d = nc.dram_tensor("wpat", [128, INDIRECT_DIM], bf16, kind="ExternalInput")
    out_d = nc.dram_tensor("out", [128, OUTPUT_DIM], f32, kind="ExternalOutput")

    rgroups = [[g * DGROUPS + i for i in range(DGROUPS)] for g in range(BGROUPS)]
    GCH = 2 * GDIMS * NCOLS  # table cols per group chunk-set
    # SP-pipe (ScalarE+Pool) builds these 4 (g,h) pairs; DVE builds the other
    # 12. PE interleaves ~3 DVE-built dim-halves with 1 SP-built one so the
    # per-segment consumption rate (0.96us/dh) stays below DVE's production
    # rate (1.19us/dh) once the SP stream fills the gap.
    SPP = [(7, 0), (7, 1), (6, 1)]
    DVEP = [(0, 0), (0, 1), (1, 0), (1, 1), (2, 0), (2, 1), (3, 0), (3, 1),
            (4, 0), (4, 1), (5, 0), (5, 1), (6, 0)]  # production order; (6,0) last

    with tile.TileContext(nc) as tc:
        with (
            tc.tile_pool(name="const", bufs=1) as cpool,
            tc.tile_pool(name="tbl", bufs=4) as tpool,
            tc.tile_pool(name="tble", bufs=1) as tepool,
            tc.tile_pool(name="xin", bufs=1) as xpool,
            tc.tile_pool(name="xrep", bufs=6) as xrpool,
            tc.tile_pool(name="w1", bufs=2) as w1pool,
            tc.tile_pool(name="W", bufs=12) as wpool,
            tc.tile_pool(name="We", bufs=1) as wepool,
            tc.tile_pool(name="sp", bufs=10) as spool,
            tc.tile_pool(name="acc", bufs=1, space="PSUM") as psumpool,
            tc.tile_pool(name="part", bufs=1, space="SBUF") as partpool,
            tc.tile_pool(name="fin", bufs=1) as finpool,
            tc.tile_pool(name="dram", bufs=2, space="DRAM") as dpool,
        ):
            # ---- constants (outside timing loop) ----
            bias_s = cpool.tile([128, 16], f32, tag="bias")
            nc.scalar.dma_start(bias_s[:], bias_d[:])
            wpat_s = cpool.tile([128, INDIRECT_DIM], bf16, tag="wpat")
            nc.scalar.dma_start(wpat_s[:], wpat_d[:])

            loop_cm = tc.For_i(0, loop_reps, 1) if loop_reps else contextlib.nullcontext()
            with loop_cm:
              for _rep in range(reps):
                # Table chunks stream on the DMA queues in PE-consumption
                # order, interleaved with x broadcasts (DRAM partition-
                # stride-0) across both HWDGE queues. Group 0's broadcast is
                # split per-di so DVE's first input lands fast.
                xrs, tqs = {}, {}

                def bcast(g, eng, split=False):
                    xr = xrpool.tile([128, GDIMS * BL], i16, tag="xrep", name=f"xr{g}")
                    if split:
                        for di in range(GDIMS):
                            src = bass.AP(xt_d, 0, [[0, 128], [1, BL]])
                            src.offset = (g * GDIMS + di) * BL
                            eng.dma_start(xr[:, di * BL:(di + 1) * BL], src)
                    else:
                        src = bass.AP(xt_d, 0, [[0, 128], [1, GDIMS * BL]])
                        src.offset = g * GDIMS * BL
                        eng.dma_start(xr[:], src)
                    xrs[g] = xr

                def tqload(g, eng, lo=0, hi=GCH):
                    tq = tqs.get(g)
                    if tq is None:
                        tq = tpool.tile([128, GCH], bf16, tag="tbl", name=f"tq{g}")
                        tqs[g] = tq
                    eng.dma_start(tq[:, lo:hi], tmain_d[:, g * GCH + lo:g * GCH + hi])
                    return tq

                # sync queue feeds DVE/PE first inputs; scalar queue feeds the
                # SP-pipe (xr7 split) then the rest, in consumption order.
                tq0a = tpool.tile([128, 2 * NCOLS], bf16, tag="tbl0", name="tq0a", bufs=1)
                # interleave x slices and group-0 table per-di on sync so the
                # first few dim-halves' inputs all land within ~5us
                xr0 = xrpool.tile([128, GDIMS * BL], i16, tag="xrep", name="xr0")
                xrs[0] = xr0
                tq0 = tpool.tile([128, GCH], bf16, tag="tbl", name="tq0")
                tqs[0] = tq0

                def xr0part(di):
                    src = bass.AP(xt_d, 0, [[0, 128], [1, BL]])
                    src.offset = di * BL
                    nc.sync.dma_start(xr0[:, di * BL:(di + 1) * BL], src)

                # ALL loads on the sync queue (a pure DMA sequencer that can
                # freely block on slot releases) in global need order: the DMA
                # engines are near-serial, so issue order decides arrivals.
                xr7 = xrpool.tile([128, GDIMS * BL], i16, tag="xrep", name="xr7")
                xrs[7] = xr7
                xr0part(0)
                src = bass.AP(xt_d, 0, [[0, 128], [1, BL]])
                src.offset = 7 * GDIMS * BL
                nc.sync.dma_start(xr7[:, 0:BL], src)
                nc.sync.dma_start(tq0a[:], tmain_d[:, 0:2 * NCOLS])
                xr0part(1)
                xr0part(2)
                xr0part(3)
                nc.sync.dma_start(tq0[:, 2 * NCOLS:4 * NCOLS],
                                  tmain_d[:, 2 * NCOLS:4 * NCOLS])
                nc.sync.dma_start(tq0[:, 4 * NCOLS:6 * NCOLS],
                                  tmain_d[:, 4 * NCOLS:6 * NCOLS])
                nc.sync.dma_start(tq0[:, 6 * NCOLS:8 * NCOLS],
                                  tmain_d[:, 6 * NCOLS:8 * NCOLS])
                src = bass.AP(xt_d, 0, [[0, 128], [1, 3 * BL]])
                src.offset = 7 * GDIMS * BL + BL
                nc.sync.dma_start(xr7[:, BL:], src)
                tq7 = tpool.tile([128, GCH], bf16, tag="tbl7", name="tq7", bufs=1)
                tqs[7] = tq7
                nc.sync.dma_start(tq7[:], tmain_d[:, 7 * GCH:8 * GCH])
                bcast(1, nc.sync)
                tqload(1, nc.sync, hi=2 * NCOLS)
                tqload(1, nc.sync, lo=2 * NCOLS)
                xe_s = xpool.tile([EP, BL], i16, tag="xe")
                nc.sync.dma_start(xe_s[:], xe_d[:])
                bcast(2, nc.sync)
                tqload(2, nc.sync)
                bcast(6, nc.sync)
                tqload(3, nc.sync)
                bcast(3, nc.sync)
                tqload(4, nc.sync)
                bcast(4, nc.sync)
                tqload(6, nc.sync)
                bcast(5, nc.sync)
                tqload(5, nc.sync)
                te_s = tepool.tile([EP, NCOLS], bf16, tag="Te")
                nc.sync.dma_start(te_s[:], tedge_d[:])

                # ---- SP-pipe: W for SPP pairs on ScalarE + Pool, di-granular
                # v = |256x - (r-1)|; W = (K1*relu(2-v))^3 - (K2*relu(1-v))^3
                Wsp = {}
                for g, h in SPP:
                    W7 = wpool.tile([128, GDIMS * BL], bf16, tag="W", name=f"Wsp{g}_{h}")
                    for di in range(GDIMS):
                        sl = slice(di * BL, (di + 1) * BL)
                        nm = f"{g}_{h}_{di}"
                        d7 = spool.tile([128, BL], f32, tag="sp", name=f"d{nm}")
                        nc.scalar.activation(d7[:], xrs[g][:, sl], AF.Abs,
                                             bias=bias_s[:, 12 + h:13 + h], scale=SCALE / XQ)
                        r1 = spool.tile([128, BL], f32, tag="sp", name=f"r1_{nm}")
                        nc.scalar.activation(r1[:], d7[:], AF.Relu,
                                             bias=bias_s[:, 14:15], scale=-K1)
                        s1 = spool.tile([128, BL], f32, tag="sp", name=f"s1_{nm}")
                        nc.scalar.activation(s1[:], r1[:], AF.Square)
                        r2 = spool.tile([128, BL], f32, tag="sp", name=f"r2_{nm}")
                        nc.scalar.activation(r2[:], d7[:], AF.Relu,
                                             bias=bias_s[:, 15:16], scale=-K2)
                        s2 = spool.tile([128, BL], f32, tag="sp", name=f"s2_{nm}")
                        nc.scalar.activation(s2[:], r2[:], AF.Square)
                        t1 = spool.tile([128, BL], f32, tag="sp", name=f"t1_{nm}")
                        nc.gpsimd.tensor_tensor(out=t1[:], in0=r1[:], in1=s1[:],
                                                op=mybir.AluOpType.mult)
                        t2 = spool.tile([128, BL], f32, tag="sp", name=f"t2_{nm}")
                        nc.gpsimd.tensor_tensor(out=t2[:], in0=r2[:], in1=s2[:],
                                                op=mybir.AluOpType.mult)
                        nc.gpsimd.tensor_tensor(out=W7[:, sl], in0=t1[:], in1=t2[:],
                                                op=mybir.AluOpType.subtract)
                    Wsp[(g, h)] = W7

                # ---- DVE W build: di-granular [128, 512] tiles throughout;
                #      edge rows after group 1 ----
                Wg = {}

                def build_dve(g, h, di):
                    lo = di * BL
                    Ws = wpool.tile([128, BL], bf16, tag="Ws", name=f"W{g}_{h}_{di}")
                    w1 = w1pool.tile([128, BL], bf16, tag="w1")
                    nc.vector._custom_dve(
                        OP_A, out=w1[:], in0=xrs[g][:, lo:lo + BL],
                        s0=bias_s[:, 4 * h:4 * h + 1], s1=bias_s[:, 4 * h + 1:4 * h + 2],
                        imm2=SCALE * K1 / XQ)
                    nc.vector._custom_dve(
                        OP_B, out=Ws[:], in0=xrs[g][:, lo:lo + BL], in1=w1[:],
                        s0=bias_s[:, 4 * h + 2:4 * h + 3], s1=bias_s[:, 4 * h + 3:4 * h + 4],
                        imm2=SCALE * K2 / XQ)
                    Wg[(g, h, di)] = Ws

                for g, h in DVEP:
                    for di in range(GDIMS):
                        build_dve(g, h, di)
                    if (g, h) == (1, 1):
                        w1e = w1pool.tile([EP, BL], bf16, tag="w1e", name="w1e")
                        nc.vector._custom_dve(
                            OP_A, out=w1e[:], in0=xe_s[:],
                            s0=bias_s[:EP, 8:9], s1=bias_s[:EP, 9:10], imm2=SCALE * K1 / XQ)
                        We = wepool.tile([EP, BL], bf16, tag="We", name="We")
                        nc.vector._custom_dve(
                            OP_B, out=We[:], in0=xe_s[:], in1=w1e[:],
                            s0=bias_s[:EP, 10:11], s1=bias_s[:EP, 11:12], imm2=SCALE * K2 / XQ)

                def wslice(g, h, di):
                    if (g, h) in Wsp:
                        return Wsp[(g, h)][:, di * BL:(di + 1) * BL]
                    return Wg[(g, h, di)][:, 0:BL]

                def tslice(g, h, di):
                    if g == 0 and di == 0:
                        return tq0a[:, h * NCOLS:(h + 1) * NCOLS]
                    base = (2 * di + h) * NCOLS
                    return tqs[g][:, base:base + NCOLS]

                # ---- main matmuls: psum chunk c accumulates [CW[c], 512] ----
                # lhsT = table chunk (stationary), rhs = W dim-half (moving).
                # Consumption interleaves SP-built dim-halves (1 per 4 DVE) so
                # PE's rate stays below DVE's; (6,0) built last goes very last.
                # Per dh: stationary = W [128 rows, 128-batch m-slice], moving
                # = table [128, 576] (two matmuls: 512 + 64 cols).
                psums = [psumpool.tile([128, NCOLS], f32, tag=f"acc{m}", name=f"ps{m}")
                         for m in range(MBL)]
                dl = [(g, h, di) for g, h in DVEP for di in range(GDIMS)]
                sl_ = [(g, h, di) for g, h in SPP for di in range(GDIMS)]
                ND = len(dl) - GDIMS  # DVE dhs before (6,0)
                # front: SP inserts ramp in at the SP pipe's production rate,
                # then steady 4 DVE + 1 SP keeps PE slower than both producers
                seq = dl[0:5] + [sl_[0]] + dl[5:7] + [sl_[1]] + dl[7:10] + [sl_[2]]
                dp = 10
                for s in sl_[3:]:
                    seq.extend(dl[dp:min(dp + 4, ND)])
                    seq.append(s)
                    dp += 4
                seq.extend(dl[min(dp, ND):ND])
                # edge rows accumulate here (before the final dim-halves)
                EDGE_AT = len(seq)
                seq.extend(dl[ND:])  # (6,0) — the last-produced DVE dhs

                for i, (g, h, di) in enumerate(seq):
                    if i == EDGE_AT:
                        for m in range(MBL):
                            lhsT = We[:, 128 * m:128 * (m + 1)]
                            nc.tensor.matmul(psums[m][:, 0:512], lhsT,
                                             te_s[:, 0:512], start=False, stop=False)
                            nc.tensor.matmul(psums[m][:, 512:NCOLS], lhsT,
                                             te_s[:, 512:NCOLS], start=False, stop=False)
                    W = wslice(g, h, di)
                    rhs = tslice(g, h, di)
                    first = i == 0
                    last = i == len(seq) - 1
                    for m in range(MBL):
                        lhsT = W[:, 128 * m:128 * (m + 1)]
                        nc.tensor.matmul(psums[m][:, 0:512], lhsT, rhs[:, 0:512],
                                         start=first, stop=last)
                        nc.tensor.matmul(psums[m][:, 512:NCOLS], lhsT, rhs[:, 512:NCOLS],
                                         start=first, stop=last)

                # ---- drain psum -> bf16 partials -> DRAM pin [512, 576] ----
                pin = dpool.tile([BL, NCOLS], bf16, tag="pin")
                for m in range(MBL):
                    P = partpool.tile([128, NCOLS], bf16, tag=f"part{m}", name=f"P{m}")
                    if m % 2 == 0:
                        nc.scalar.copy(out=P[:], in_=psums[m][:])
                    else:
                        nc.vector.tensor_scalar(out=P[:], in0=psums[m][:],
                                                scalar1=1.0, scalar2=None,
                                                op0=mybir.AluOpType.mult)
                    nc.sync.dma_start(pin[128 * m:128 * (m + 1), :], P[:])

                if "rs" in skip:
                    zsrc = pin[0:128, :]
                else:
                    pout = dpool.tile([128, NCOLS], bf16, tag="pout")
                    nc.gpsimd.collective_compute(
                        "ReduceScatter", mybir.AluOpType.add,
                        replica_groups=rgroups,
                        ins=[pin.opt()], outs=[pout.opt()],
                    )
                    zsrc = pout[:]

                # ---- tail: z [128 batch, 576] -> exp / weighted-reduce ----
                zb = finpool.tile([128, NCOLS], bf16, tag="zb")
                nc.scalar.dma_start(zb[:], zsrc)
                E = finpool.tile([128, INDIRECT_DIM], bf16, tag="E")
                Em = finpool.tile([128, INDIRECT_DIM], bf16, tag="Em")
                red = finpool.tile([128, OUTPUT_DIM], f32, tag="red")
                HIND = INDIRECT_DIM // 2
                for hh in range(2):
                    cs = slice(hh * HIND, (hh + 1) * HIND)
                    nc.scalar.activation(E[:, cs], zb[:, cs], AF.Exp)
                    nc.vector.tensor_tensor(out=Em[:, cs], in0=E[:, cs],
                                            in1=wpat_s[:, cs],
                                            op=mybir.AluOpType.mult)
                    nc.vector.tensor_reduce(
                        out=red[:, hh * 32:(hh + 1) * 32],
                        in_=Em[:, cs].rearrange("p (a b) -> p a b", b=2 * NUM_EXPS),
                        axis=mybir.AxisListType.X, op=mybir.AluOpType.add)
                zd = finpool.tile([128, OUTPUT_DIM], f32, tag="zd")
                nc.scalar.copy(out=zd[:], in_=zb[:, INDIRECT_DIM:NCOLS])
                fin = finpool.tile([128, OUTPUT_DIM], f32, tag="fin")
                nc.vector.tensor_tensor(out=fin[:], in0=red[:], in1=zd[:],
                                        op=mybir.AluOpType.add)
                nc.scalar.dma_start(out_d[:], fin[:])

    nc.compile()
    if not skip and not loop_reps and reps == 1:
        _NC = nc
    return nc


def _shard_inputs(x, direct_table, indirect_table):
    import ml_dtypes
    bf16 = ml_dtypes.bfloat16
    comb = np.concatenate([indirect_table, direct_table], axis=1)  # [33152, 576]
    bias = _bias_consts()
    wpat = _wpat_const()
    in_maps = []
    for c in range(NCORES):
        bg, dg = c // DGROUPS, c % DGROUPS
        dims = range(DPC * dg, DPC * (dg + 1))
        bsl = slice(BL * bg, BL * (bg + 1))
        xt = np.ascontiguousarray(x[bsl, DPC * dg:DPC * (dg + 1)].T)  # [32, 512]
        if X_I16:
            xt = np.round(xt * XQ).astype(np.int16)
        xe = np.repeat(xt, EDGE_ROWS, axis=0)  # [96, 512]
        tmain = np.concatenate(
            [comb[d * DENSITY:d * DENSITY + MAIN_ROWS] for d in dims], axis=0)
        tmain = tmain.reshape(2 * DPC, 128, NCOLS).transpose(1, 0, 2).reshape(128, 2 * DPC * NCOLS)
        tedge = np.concatenate(
            [comb[d * DENSITY + MAIN_ROWS:(d + 1) * DENSITY] for d in dims], axis=0)
        in_maps.append({
            "xt": np.ascontiguousarray(xt),
            "xe": np.ascontiguousarray(xe),
            "tmain": np.ascontiguousarray(tmain).astype(bf16),
            "tedge": np.ascontiguousarray(tedge).astype(bf16),
            "bias": bias,
            "wpat": wpat.astype(bf16),
        })
    return in_maps


def kernel(x, direct_table, indirect_table):
    from concourse.bass_utils import run_bass_kernel_spmd
    x = np.asarray(x, np.float32)
    direct_table = np.asarray(direct_table, np.float32)
    indirect_table = np.asarray(indirect_table, np.float32)
    assert x.shape == (BATCH, INPUT_DIM)
    nc = _build()
    in_maps = _shard_inputs(x, direct_table, indirect_table)
    res = run_bass_kernel_spmd(nc, in_maps, core_ids=list(range(NCORES)))
    # core (bg, dg) returns out [128, 64] for batch rows 512*bg + 128*dg
    return np.concatenate([r["out"] for r in res.results], axis=0)


# revision 52
# speedup vs baseline: 1.3138x; 1.3138x over previous
# Bass/Trainium2 kernel for nn_ABELSpline (embedding_lookup via implicit one-hot matmul).
#
# Math: out[b,o] = sum_d sum_r B(256*x[b,d] + 3 - r) * table[d*259 + r, o]
#   where B is the cubic B-spline basis on [0,4). The gather/multiply/reduce of the
#   reference is exactly a dense matmul with W[r, b] = B(...) built on the fly:
#     B(y) = relu(2-u)^3/6 - (2/3)*relu(1-u)^3,  u = |y-2|
#
# Orientation: stationary = table chunk [128 rows, <=128 cols], moving = W
# [128 rows, 512 batch]; psum chunk c accumulates [cols_c, 512 batch] over all
# 64 dim-halves + edge rows. 293 matmuls of N=512 (vs ~1040 in the
# batch-stationary orientation) cut PE sequencer issue work nearly in half
# while keeping PE engine time at the dense-matmul floor (~62us).
#
# W build (per core, 16 (g,h) pairs of [128 rows, 2048]):
#   - 13 pairs on DVE: two fused custom ops (relu-min cube) -> ~58us
#   - 3 pairs ((7,0),(7,1),(6,1)) on ScalarE activations + Pool combines
# Tables are pre-converted to bf16 on host.
#
# Sharding: cores form a 2x4 grid: batch halves (512) x dim quarters (32 dims).
# Partials [576 cols, 512 batch] bf16 ReduceScatter (cc_dim=Free) over the 4
# dim-parallel cores -> each core owns a 128-batch slice [576, 128]; the
# anti-symmetric-exponential tail is exp on ScalarE + a [128,64] select-matmul
# partition reduction on PE; each core outputs out^T [64, 128] f32 and the
# host transposes/assembles.

import numpy as np

BATCH = 1024
INPUT_DIM = 128
DENSITY = 259
OUTPUT_DIM = 64
NUM_EXPS = 4
INDIRECT_DIM = 2 * NUM_EXPS * OUTPUT_DIM  # 512
NCOLS = INDIRECT_DIM + OUTPUT_DIM  # 576
NCORES = 8
DGROUPS = 4                 # dim-parallel cores per batch half
BGROUPS = NCORES // DGROUPS  # 2 batch halves
DPC = INPUT_DIM // DGROUPS  # dims per core = 32
BL = BATCH // BGROUPS       # local batch = 512
GDIMS = 4                   # dims per W build group
NGROUPS = DPC // GDIMS      # 8
MAIN_ROWS = 256
EDGE_ROWS = DENSITY - MAIN_ROWS  # 3
EP = DPC * EDGE_ROWS        # 96 edge partitions
SCALE = float(DENSITY - 3)  # 256
K1 = float((1.0 / 6.0) ** (1.0 / 3.0))
K2 = float((4.0 / 6.0) ** (1.0 / 3.0))
MBL = BL // 128             # psum batch chunks = 4
X_I16 = True                # ship x as int16 round(x * XQ) (halves x DMA)
XQ = 32767.0 if X_I16 else 1.0

_OPS = None
_NC = None


def _register_ops():
    global _OPS
    if _OPS is not None:
        return _OPS
    from concourse.dve_spec import Spec, Src0, Src1, C0, C1, C2, Bin, relu, sq, minn, lower
    from concourse.dve_uop import AluOp, DveOpSpec
    from concourse import dve_ops
    from concourse.dve_ops import DveOp

    def relu3_body():
        # relu(min(C0 - m, m + C1))^3 with m = Src0*C2
        m = Src0 * C2
        a = Bin(AluOp.SUBTRACT, C0, m)
        b = Bin(AluOp.ADD, m, C1)
        r = relu(minn(a, b))
        return r * sq(r)

    def ref_a(in0, in1, s0, s1, imm2):
        t = np.maximum(np.minimum(s0 - in0 * imm2, in0 * imm2 + s1), 0.0)
        return (t ** 3).astype(np.float32)

    def ref_b(in0, in1, s0, s1, imm2):
        t = np.maximum(np.minimum(s0 - in0 * imm2, in0 * imm2 + s1), 0.0)
        return (in1 - t ** 3).astype(np.float32)

    spec_a = Spec(body=relu3_body(), reference=ref_a)
    spec_b = Spec(body=Bin(AluOp.SUBTRACT, Src1, relu3_body()), reference=ref_b)

    ops = []
    for name, spec in (("SPLINE3A_ANT", spec_a), ("SPLINE3B_ANT", spec_b)):
        if name in dve_ops._SUB_OPCODE_FOR_NAME:
            ops.append(next(o for o in dve_ops.OPS if o.name == name))
            continue
        row = max(dve_ops._SUB_OPCODE_FOR_NAME.values()) + 1
        assert row < 0x20
        uops = lower(spec, ver="v3")
        rd1 = name.endswith("B_ANT")
        sha = DveOpSpec(name=name, opcode=row, uops=uops, rd1_en=rd1).sha("v3")
        op = DveOp(name, spec, subdim=False, uops_sha={"v3": sha})
        dve_ops.OPS.append(op)
        dve_ops._SUB_OPCODE_FOR_NAME[name] = row
        dve_ops.CUSTOM_DVE_SPECS[name] = spec
        ops.append(op)
    _OPS = tuple(ops)
    return _OPS


def _bias_consts():
    # [128, 16] f32 per-partition scalars.
    # cols 4h+{0..3}: main half h (r = 128h + p): K1*(r+1), K1*(3-r), K2*r, K2*(2-r)
    # cols 8..11: edge (r = 256 + p%3)
    # cols 12/13: SP-pipe Abs bias 1-r for half h (arg = 256x - (r-1))
    b = np.zeros((128, 16), np.float32)
    p = np.arange(128, dtype=np.float64)
    for h in (0, 1):
        r = 128 * h + p
        b[:, 4 * h + 0] = K1 * (r + 1)
        b[:, 4 * h + 1] = K1 * (3 - r)
        b[:, 4 * h + 2] = K2 * r
        b[:, 4 * h + 3] = K2 * (2 - r)
        b[:, 12 + h] = 1.0 - r
    b[:, 14] = 2.0 * K1
    b[:, 15] = K2
    re = 256 + (p % 3)
    b[:, 8] = K1 * (re + 1)
    b[:, 9] = K1 * (3 - re)
    b[:, 10] = K2 * re
    b[:, 11] = K2 * (2 - re)
    return b


def _wpat_const():
    w = np.array([1.0 / (i + 1) ** 2 for i in range(NUM_EXPS)], np.float32)
    row = np.concatenate([w, -w])  # [8]
    return np.tile(row, (128, OUTPUT_DIM)).astype(np.float32)  # [128, 512]


def _build(skip=(), loop_reps=0, reps=1):
    global _NC
    if _NC is not None and not skip and not loop_reps and reps == 1:
        return _NC
    import contextlib
    import concourse.bass as bass
    import concourse.bacc as bacc
    import concourse.tile as tile
    import concourse.mybir as mybir

    OP_A, OP_B = _register_ops()
    f32 = mybir.dt.float32
    bf16 = mybir.dt.bfloat16
    AF = mybir.ActivationFunctionType

    nc = bacc.Bacc("TRN2", target_bir_lowering=False, debug=False, num_devices=NCORES)

    i16 = mybir.dt.int16 if X_I16 else mybir.dt.float32
    xt_d = nc.dram_tensor("xt", [DPC, BL], i16, kind="ExternalInput")
    xe_d = nc.dram_tensor("xe", [EP, BL], i16, kind="ExternalInput")
    # tmain pre-permuted on host (bf16): [128, (2*DPC)*NCOLS], chunk k=2*di+h at cols k*NCOLS
    tmain_d = nc.dram_tensor("tmain", [128, 2 * DPC * NCOLS], bf16, kind="ExternalInput")
    tedge_d = nc.dram_tensor("tedge", [EP, NCOLS], bf16, kind="ExternalInput")
    bias_d = nc.dram_tensor("bias", [128, 16], f32, kind="ExternalInput")
    wpat_d = nc.dram_tensor("wpat", [128, INDIRECT_DIM], bf16, kind="ExternalInput")
    out_d = nc.dram_tensor("out", [128, OUTPUT_DIM], f32, kind="ExternalOutput")

    rgroups = [[g * DGROUPS + i for i in range(DGROUPS)] for g in range(BGROUPS)]
    GCH = 2 * GDIMS * NCOLS  # table cols per group chunk-set
    # SP-pipe (ScalarE+Pool) builds these 4 (g,h) pairs; DVE builds the other
    # 12. PE interleaves ~3 DVE-built dim-halves with 1 SP-built one so the
    # per-segment consumption rate (0.96us/dh) stays below DVE's production
    # rate (1.19us/dh) once the SP stream fills the gap.
    SPP = [(7, 0), (7, 1), (6, 1)]
    DVEP = [(0, 0), (0, 1), (1, 0), (1, 1), (2, 0), (2, 1), (3, 0), (3, 1),
            (4, 0), (4, 1), (5, 0), (5, 1), (6, 0)]  # production order; (6,0) last

    with tile.TileContext(nc) as tc:
        with (
            tc.tile_pool(name="const", bufs=1) as cpool,
            tc.tile_pool(name="tbl", bufs=4) as tpool,
            tc.tile_pool(name="tble", bufs=1) as tepool,
            tc.tile_pool(name="xin", bufs=1) as xpool,
            tc.tile_pool(name="xrep", bufs=6) as xrpool,
            tc.tile_pool(name="w1", bufs=2) as w1pool,
            tc.tile_pool(name="W", bufs=12) as wpool,
            tc.tile_pool(name="We", bufs=1) as wepool,
            tc.tile_pool(name="sp", bufs=10) as spool,
            tc.tile_pool(name="acc", bufs=1, space="PSUM") as psumpool,
            tc.tile_pool(name="part", bufs=1, space="SBUF") as partpool,
            tc.tile_pool(name="fin", bufs=1) as finpool,
            tc.tile_pool(name="dram", bufs=2, space="DRAM") as dpool,
        ):
            # ---- constants (outside timing loop) ----
            bias_s = cpool.tile([128, 16], f32, tag="bias")
            nc.scalar.dma_start(bias_s[:], bias_d[:])
            wpat_s = cpool.tile([128, INDIRECT_DIM], bf16, tag="wpat")
            nc.scalar.dma_start(wpat_s[:], wpat_d[:])

            loop_cm = tc.For_i(0, loop_reps, 1) if loop_reps else contextlib.nullcontext()
            with loop_cm:
              for _rep in range(reps):
                # Table chunks stream on the DMA queues in PE-consumption
                # order, interleaved with x broadcasts (DRAM partition-
                # stride-0) across both HWDGE queues. Group 0's broadcast is
                # split per-di so DVE's first input lands fast.
                xrs, tqs = {}, {}

                def bcast(g, eng, split=False):
                    xr = xrpool.tile([128, GDIMS * BL], i16, tag="xrep", name=f"xr{g}")
                    if split:
                        for di in range(GDIMS):
                            src = bass.AP(xt_d, 0, [[0, 128], [1, BL]])
                            src.offset = (g * GDIMS + di) * BL
                            eng.dma_start(xr[:, di * BL:(di + 1) * BL], src)
                    else:
                        src = bass.AP(xt_d, 0, [[0, 128], [1, GDIMS * BL]])
                        src.offset = g * GDIMS * BL
                        eng.dma_start(xr[:], src)
                    xrs[g] = xr

                def tqload(g, eng, lo=0, hi=GCH):
                    tq = tqs.get(g)
                    if tq is None:
                        tq = tpool.tile([128, GCH], bf16, tag="tbl", name=f"tq{g}")
                        tqs[g] = tq
                    eng.dma_start(tq[:, lo:hi], tmain_d[:, g * GCH + lo:g * GCH + hi])
                    return tq

                # sync queue feeds DVE/PE first inputs; scalar queue feeds the
                # SP-pipe (xr7 split) then the rest, in consumption order.
                tq0a = tpool.tile([128, 2 * NCOLS], bf16, tag="tbl0", name="tq0a", bufs=1)
                # interleave x slices and group-0 table per-di on sync so the
                # first few dim-halves' inputs all land within ~5us
                xr0 = xrpool.tile([128, GDIMS * BL], i16, tag="xrep", name="xr0")
                xrs[0] = xr0
                tq0 = tpool.tile([128, GCH], bf16, tag="tbl", name="tq0")
                tqs[0] = tq0

                def xr0part(di):
                    src = bass.AP(xt_d, 0, [[0, 128], [1, BL]])
                    src.offset = di * BL
                    nc.sync.dma_start(xr0[:, di * BL:(di + 1) * BL], src)

                # Early/critical loads on the sync queue (pure DMA
                # sequencer) in need order. The late loads (incl. all
                # slot-recycled ones) go on the scalar queue but are EMITTED
                # AFTER the SP-pipe section, so a blocked slot can never
                # freeze Act's compute. Two queues ~= two DMA engine groups.
                xr7 = xrpool.tile([128, GDIMS * BL], i16, tag="xrep", name="xr7")
                xrs[7] = xr7
                xr0part(0)
                src = bass.AP(xt_d, 0, [[0, 128], [1, BL]])
                src.offset = 7 * GDIMS * BL
                nc.sync.dma_start(xr7[:, 0:BL], src)
                nc.sync.dma_start(tq0a[:], tmain_d[:, 0:2 * NCOLS])
                xr0part(1)
                xr0part(2)
                xr0part(3)
                nc.sync.dma_start(tq0[:, 2 * NCOLS:4 * NCOLS],
                                  tmain_d[:, 2 * NCOLS:4 * NCOLS])
                nc.sync.dma_start(tq0[:, 4 * NCOLS:6 * NCOLS],
                                  tmain_d[:, 4 * NCOLS:6 * NCOLS])
                nc.sync.dma_start(tq0[:, 6 * NCOLS:8 * NCOLS],
                                  tmain_d[:, 6 * NCOLS:8 * NCOLS])
                src = bass.AP(xt_d, 0, [[0, 128], [1, 3 * BL]])
                src.offset = 7 * GDIMS * BL + BL
                nc.sync.dma_start(xr7[:, BL:], src)
                tq7 = tpool.tile([128, GCH], bf16, tag="tbl7", name="tq7", bufs=1)
                tqs[7] = tq7
                nc.sync.dma_start(tq7[:], tmain_d[:, 7 * GCH:8 * GCH])
                bcast(1, nc.sync)
                tqload(1, nc.sync, hi=2 * NCOLS)
                tqload(1, nc.sync, lo=2 * NCOLS)
                xe_s = xpool.tile([EP, BL], i16, tag="xe")
                nc.sync.dma_start(xe_s[:], xe_d[:])
                bcast(6, nc.scalar)
                tqload(3, nc.sync)
                bcast(3, nc.sync)
                te_s = tepool.tile([EP, NCOLS], bf16, tag="Te")

                # ---- SP-pipe: W for SPP pairs on ScalarE + Pool, di-granular
                # v = |256x - (r-1)|; W = (K1*relu(2-v))^3 - (K2*relu(1-v))^3
                Wsp = {}
                for g, h in SPP:
                    W7 = wpool.tile([128, GDIMS * BL], bf16, tag="W", name=f"Wsp{g}_{h}")
                    for di in range(GDIMS):
                        sl = slice(di * BL, (di + 1) * BL)
                        nm = f"{g}_{h}_{di}"
                        d7 = spool.tile([128, BL], f32, tag="sp", name=f"d{nm}")
                        nc.scalar.activation(d7[:], xrs[g][:, sl], AF.Abs,
                                             bias=bias_s[:, 12 + h:13 + h], scale=SCALE / XQ)
                        r1 = spool.tile([128, BL], f32, tag="sp", name=f"r1_{nm}")
                        nc.scalar.activation(r1[:], d7[:], AF.Relu,
                                             bias=bias_s[:, 14:15], scale=-K1)
                        s1 = spool.tile([128, BL], f32, tag="sp", name=f"s1_{nm}")
                        nc.scalar.activation(s1[:], r1[:], AF.Square)
                        r2 = spool.tile([128, BL], f32, tag="sp", name=f"r2_{nm}")
                        nc.scalar.activation(r2[:], d7[:], AF.Relu,
                                             bias=bias_s[:, 15:16], scale=-K2)
                        s2 = spool.tile([128, BL], f32, tag="sp", name=f"s2_{nm}")
                        nc.scalar.activation(s2[:], r2[:], AF.Square)
                        t1 = spool.tile([128, BL], f32, tag="sp", name=f"t1_{nm}")
                        nc.gpsimd.tensor_tensor(out=t1[:], in0=r1[:], in1=s1[:],
                                                op=mybir.AluOpType.mult)
                        t2 = spool.tile([128, BL], f32, tag="sp", name=f"t2_{nm}")
                        nc.gpsimd.tensor_tensor(out=t2[:], in0=r2[:], in1=s2[:],
                                                op=mybir.AluOpType.mult)
                        nc.gpsimd.tensor_tensor(out=W7[:, sl], in0=t1[:], in1=t2[:],
                                                op=mybir.AluOpType.subtract)
                    Wsp[(g, h)] = W7

                # late loads (scalar queue; see note above)
                bcast(2, nc.scalar)
                tqload(2, nc.scalar)
                tqload(4, nc.scalar)
                bcast(4, nc.scalar)
                tqload(6, nc.scalar)
                bcast(5, nc.scalar)
                tqload(5, nc.scalar)
                nc.scalar.dma_start(te_s[:], tedge_d[:])

                # ---- DVE W build: di-granular [128, 512] tiles throughout;
                #      edge rows after group 1 ----
                Wg = {}

                def build_dve(g, h, di):
                    lo = di * BL
                    Ws = wpool.tile([128, BL], bf16, tag="Ws", name=f"W{g}_{h}_{di}")
                    w1 = w1pool.tile([128, BL], bf16, tag="w1")
                    nc.vector._custom_dve(
                        OP_A, out=w1[:], in0=xrs[g][:, lo:lo + BL],
                        s0=bias_s[:, 4 * h:4 * h + 1], s1=bias_s[:, 4 * h + 1:4 * h + 2],
                        imm2=SCALE * K1 / XQ)
                    nc.vector._custom_dve(
                        OP_B, out=Ws[:], in0=xrs[g][:, lo:lo + BL], in1=w1[:],
                        s0=bias_s[:, 4 * h + 2:4 * h + 3], s1=bias_s[:, 4 * h + 3:4 * h + 4],
                        imm2=SCALE * K2 / XQ)
                    Wg[(g, h, di)] = Ws

                for g, h in DVEP:
                    for di in range(GDIMS):
                        build_dve(g, h, di)
                    if (g, h) == (1, 1):
                        w1e = w1pool.tile([EP, BL], bf16, tag="w1e", name="w1e")
                        nc.vector._custom_dve(
                            OP_A, out=w1e[:], in0=xe_s[:],
                            s0=bias_s[:EP, 8:9], s1=bias_s[:EP, 9:10], imm2=SCALE * K1 / XQ)
                        We = wepool.tile([EP, BL], bf16, tag="We", name="We")
                        nc.vector._custom_dve(
                            OP_B, out=We[:], in0=xe_s[:], in1=w1e[:],
                            s0=bias_s[:EP, 10:11], s1=bias_s[:EP, 11:12], imm2=SCALE * K2 / XQ)

                def wslice(g, h, di):
                    if (g, h) in Wsp:
                        return Wsp[(g, h)][:, di * BL:(di + 1) * BL]
                    return Wg[(g, h, di)][:, 0:BL]

                def tslice(g, h, di):
                    if g == 0 and di == 0:
                        return tq0a[:, h * NCOLS:(h + 1) * NCOLS]
                    base = (2 * di + h) * NCOLS
                    return tqs[g][:, base:base + NCOLS]

                # ---- main matmuls: psum chunk c accumulates [CW[c], 512] ----
                # lhsT = table chunk (stationary), rhs = W dim-half (moving).
                # Consumption interleaves SP-built dim-halves (1 per 4 DVE) so
                # PE's rate stays below DVE's; (6,0) built last goes very last.
                # Per dh: stationary = W [128 rows, 128-batch m-slice], moving
                # = table [128, 576] (two matmuls: 512 + 64 cols).
                psums = [psumpool.tile([128, NCOLS], f32, tag=f"acc{m}", name=f"ps{m}")
                         for m in range(MBL)]
                dl = [(g, h, di) for g, h in DVEP for di in range(GDIMS)]
                sl_ = [(g, h, di) for g, h in SPP for di in range(GDIMS)]
                ND = len(dl) - GDIMS  # DVE dhs before (6,0)
                # front: SP inserts ramp in at the SP pipe's production rate,
                # then steady 4 DVE + 1 SP keeps PE slower than both producers
                seq = dl[0:5] + [sl_[0]] + dl[5:7] + [sl_[1]] + dl[7:10] + [sl_[2]]
                dp = 10
                for s in sl_[3:]:
                    seq.extend(dl[dp:min(dp + 4, ND)])
                    seq.append(s)
                    dp += 4
                seq.extend(dl[min(dp, ND):ND])
                # edge rows accumulate here (before the final dim-halves)
                EDGE_AT = len(seq)
                seq.extend(dl[ND:])  # (6,0) — the last-produced DVE dhs

                for i, (g, h, di) in enumerate(seq):
                    if i == EDGE_AT:
                        for m in range(MBL):
                            lhsT = We[:, 128 * m:128 * (m + 1)]
                            nc.tensor.matmul(psums[m][:, 0:512], lhsT,
                                             te_s[:, 0:512], start=False, stop=False)
                            nc.tensor.matmul(psums[m][:, 512:NCOLS], lhsT,
                                             te_s[:, 512:NCOLS], start=False, stop=False)
                    W = wslice(g, h, di)
                    rhs = tslice(g, h, di)
                    first = i == 0
                    last = i == len(seq) - 1
                    for m in range(MBL):
                        lhsT = W[:, 128 * m:128 * (m + 1)]
                        nc.tensor.matmul(psums[m][:, 0:512], lhsT, rhs[:, 0:512],
                                         start=first, stop=last)
                        nc.tensor.matmul(psums[m][:, 512:NCOLS], lhsT, rhs[:, 512:NCOLS],
                                         start=first, stop=last)

                # ---- drain psum -> bf16 partials -> DRAM pin [512, 576] ----
                pin = dpool.tile([BL, NCOLS], bf16, tag="pin")
                for m in range(MBL):
                    P = partpool.tile([128, NCOLS], bf16, tag=f"part{m}", name=f"P{m}")
                    if m % 2 == 0:
                        nc.scalar.copy(out=P[:], in_=psums[m][:])
                    else:
                        nc.vector.tensor_scalar(out=P[:], in0=psums[m][:],
                                                scalar1=1.0, scalar2=None,
                                                op0=mybir.AluOpType.mult)
                    nc.sync.dma_start(pin[128 * m:128 * (m + 1), :], P[:])

                if "rs" in skip:
                    zsrc = pin[0:128, :]
                else:
                    pout = dpool.tile([128, NCOLS], bf16, tag="pout")
                    nc.gpsimd.collective_compute(
                        "ReduceScatter", mybir.AluOpType.add,
                        replica_groups=rgroups,
                        ins=[pin.opt()], outs=[pout.opt()],
                    )
                    zsrc = pout[:]

                # ---- tail: z [128 batch, 576] -> exp / weighted-reduce ----
                zb = finpool.tile([128, NCOLS], bf16, tag="zb")
                nc.scalar.dma_start(zb[:], zsrc)
                E = finpool.tile([128, INDIRECT_DIM], bf16, tag="E")
                Em = finpool.tile([128, INDIRECT_DIM], bf16, tag="Em")
                red = finpool.tile([128, OUTPUT_DIM], f32, tag="red")
                HIND = INDIRECT_DIM // 2
                for hh in range(2):
                    cs = slice(hh * HIND, (hh + 1) * HIND)
                    nc.scalar.activation(E[:, cs], zb[:, cs], AF.Exp)
                    nc.vector.tensor_tensor(out=Em[:, cs], in0=E[:, cs],
                                            in1=wpat_s[:, cs],
                                            op=mybir.AluOpType.mult)
                    nc.vector.tensor_reduce(
                        out=red[:, hh * 32:(hh + 1) * 32],
                        in_=Em[:, cs].rearrange("p (a b) -> p a b", b=2 * NUM_EXPS),
                        axis=mybir.AxisListType.X, op=mybir.AluOpType.add)
                zd = finpool.tile([128, OUTPUT_DIM], f32, tag="zd")
                nc.scalar.copy(out=zd[:], in_=zb[:, INDIRECT_DIM:NCOLS])
                fin = finpool.tile([128, OUTPUT_DIM], f32, tag="fin")
                nc.vector.tensor_tensor(out=fin[:], in0=red[:], in1=zd[:],
                                        op=mybir.AluOpType.add)
                nc.scalar.dma_start(out_d[:], fin[:])

    nc.compile()
    if not skip and not loop_reps and reps == 1:
        _NC = nc
    return nc


def _shard_inputs(x, direct_table, indirect_table):
    import ml_dtypes
    bf16 = ml_dtypes.bfloat16
    comb = np.concatenate([indirect_table, direct_table], axis=1)  # [33152, 576]
    bias = _bias_consts()
    wpat = _wpat_const()
    in_maps = []
    for c in range(NCORES):
        bg, dg = c // DGROUPS, c % DGROUPS
        dims = range(DPC * dg, DPC * (dg + 1))
        bsl = slice(BL * bg, BL * (bg + 1))
        xt = np.ascontiguousarray(x[bsl, DPC * dg:DPC * (dg + 1)].T)  # [32, 512]
        if X_I16:
            xt = np.round(xt * XQ).astype(np.int16)
        xe = np.repeat(xt, EDGE_ROWS, axis=0)  # [96, 512]
        tmain = np.concatenate(
            [comb[d * DENSITY:d * DENSITY + MAIN_ROWS] for d in dims], axis=0)
        tmain = tmain.reshape(2 * DPC, 128, NCOLS).transpose(1, 0, 2).reshape(128, 2 * DPC * NCOLS)
        tedge = np.concatenate(
            [comb[d * DENSITY + MAIN_ROWS:(d + 1) * DENSITY] for d in dims], axis=0)
        in_maps.append({
            "xt": np.ascontiguousarray(xt),
            "xe": np.ascontiguousarray(xe),
            "tmain": np.ascontiguousarray(tmain).astype(bf16),
            "tedge": np.ascontiguousarray(tedge).astype(bf16),
            "bias": bias,
            "wpat": wpat.astype(bf16),
        })
    return in_maps


def kernel(x, direct_table, indirect_table):
    from concourse.bass_utils import run_bass_kernel_spmd
    x = np.asarray(x, np.float32)
    direct_table = np.asarray(direct_table, np.float32)
    indirect_table = np.asarray(indirect_table, np.float32)
    assert x.shape == (BATCH, INPUT_DIM)
    nc = _build()
    in_maps = _shard_inputs(x, direct_table, indirect_table)
    res = run_bass_kernel_spmd(nc, in_maps, core_ids=list(range(NCORES)))
    # core (bg, dg) returns out [128, 64] for batch rows 512*bg + 128*dg
    return np.concatenate([r["out"] for r in res.results], axis=0)
